# revision 7
# baseline (speedup 1.0000x reference)
"""Trainium2 Bass kernel for seq-first MultiHeadAttention (S=2048, B=2, D=1024, H=16).

Sharding: batch*head parallel across 8 cores. Core c handles batch b = c//4 and
heads 4*(c%4) .. 4*(c%4)+3 (4 heads). Q/K/V/O projection weights are split
per-head (tensor parallel); the W_o row-parallel partial outputs are summed on
the host during unshard.

Device-side layout (per core):
  - inputs pre-transposed on host: xqT/xkT/xvT (D, S), wqT/wkT/wvT (D, 4*dk),
    woT (4*dk, D), maskT (S, S) uint8  [maskT[k,q] = mask[q,k] & kpm[b,k]]
  - QT/KT computed d-major (4*dk, S); V computed k-major (S, 4*dk+ones)
  - scoresT[k,q] = K @ Q^T per head; P = exp(scale*scoresT) * maskT  (no max
    subtraction: scores are O(1); masked entries exactly 0, matching the
    reference where exp(-1e9 - max) underflows to 0)
  - AV with an appended ones-column computes both head_out^T and the softmax
    row sums in one pass; normalization applied to head_out^T before W_o
  - outputs: attnT (4, S, S) unnormalized-transposed probs, recip (4, S)
    reciprocal row sums, outp (S, D) per-core partial of the final projection
Host unshard: attn = (attnT * recip).T per head; out = sum_b-cores(outp) + bo.
"""

import math
import sys

import numpy as np

sys.path.insert(0, "/opt/trn_rl_repo")

S, B, D, H = 2048, 2, 1024, 16
DK = 64
HPC = 4  # heads per core
N_CORES = 8
F32R = True  # use fp32r (11-bit mantissa, 4x matmul throughput) for matmuls


def round_f32r(x):
    """Round f32 array to fp32r (f32 layout, low 12 mantissa bits rounded away)."""
    u = np.ascontiguousarray(x, np.float32).view(np.uint32)
    r = ((u.astype(np.uint64) + 0x800) & 0xFFFFF000).astype(np.uint32)
    return r.view(np.float32)


def spill_waits(nc, max_waits=1):
    """Move excess per-instruction sem waits onto standalone EventSemaphore
    instructions (this walrus build rejects >1 SyncWait per instruction)."""
    import concourse.mybir as mybir

    n = 0
    for func in nc.m.functions:
        for bb in func.blocks:
            out = []
            changed = False
            for inst in bb.instructions:
                si = inst.sync_info
                if si is not None and len(si.on_wait) > max_waits:
                    waits = list(si.on_wait)
                    spill, keep = waits[:-max_waits], waits[-max_waits:]
                    for w in spill:
                        ev = mybir.InstEventSemaphore(
                            name=f"wspill-{nc.next_id()}", ins=[], outs=[]
                        )
                        ev.engine = inst.engine
                        ev.sync_info = mybir.SyncInfo(on_wait=[w], on_update=[])
                        out.append(ev)
                        n += 1
                    inst.sync_info = mybir.SyncInfo(
                        on_wait=keep, on_update=list(si.on_update)
                    )
                    changed = True
                out.append(inst)
            if changed:
                bb.instructions = out
    return n


def build_nc(s=S, d=D, hpc=HPC, dk=DK, f32r=True, spill=True):
    """Emit the single-core SPMD Bass program (parameterized for small-scale sim)."""
    from contextlib import ExitStack

    import concourse.bass as bass
    import concourse.mybir as mybir
    import concourse.tile as tile

    f32 = mybir.dt.float32
    mf = mybir.dt.float32r if f32r else mybir.dt.float32
    u8 = mybir.dt.uint8
    Exp = mybir.ActivationFunctionType.Exp

    assert s % 512 == 0 and d % 256 == 0 and (hpc * dk) % 128 == 0
    KT = s // 128  # k tiles
    NQC = s // 512  # 512-wide q chunks
    ICH = d // 128  # input-feature chunks
    DT = (hpc * dk) // 128  # stacked-head tiles for QT/KT (2 heads per tile)
    QW = min(1024, s)  # projection psum width
    QH = s // QW
    NOC = d // 512  # output projection chunks
    VC = dk + 1  # per-head V columns incl. ones column
    scale = 1.0 / math.sqrt(dk)

    nc = bass.Bass("TRN2", target_bir_lowering=False, debug=False)

    xqT_d = nc.dram_tensor("xqT", [d, s], mf, kind="ExternalInput")
    xkT_d = nc.dram_tensor("xkT", [d, s], mf, kind="ExternalInput")
    xvT_d = nc.dram_tensor("xvT", [d, s], mf, kind="ExternalInput")
    wqT_d = nc.dram_tensor("wqT", [d, hpc * dk], mf, kind="ExternalInput")
    wkT_d = nc.dram_tensor("wkT", [d, hpc * dk], mf, kind="ExternalInput")
    wvT_d = nc.dram_tensor("wvT", [d, hpc * dk], mf, kind="ExternalInput")
    woT_d = nc.dram_tensor("woT", [hpc * dk, d], mf, kind="ExternalInput")
    maskT_d = nc.dram_tensor("maskT", [s, s], u8, kind="ExternalInput")
    vtpl_d = nc.dram_tensor("vtpl", [128, hpc * VC], mf, kind="ExternalInput")
    attnT_d = nc.dram_tensor("attnT", [hpc, s, s], mf, kind="ExternalOutput")
    recip_d = nc.dram_tensor("recip", [hpc, s], f32, kind="ExternalOutput")
    outp_d = nc.dram_tensor("outp", [s, d], f32, kind="ExternalOutput")

    with tile.TileContext(nc) as tc, ExitStack() as ctx:
        # ---- persistent pools (live across phases)
        qtkt_pool = ctx.enter_context(tc.tile_pool(name="qtkt", bufs=1))
        v_pool = ctx.enter_context(tc.tile_pool(name="vkm", bufs=1))
        hn_pool = ctx.enter_context(tc.tile_pool(name="hn", bufs=1))
        mask_pool = ctx.enter_context(tc.tile_pool(name="mask", bufs=1))
        ones_pool = ctx.enter_context(tc.tile_pool(name="ones", bufs=1))

        ones64 = ones_pool.tile([1, dk], f32, name="ones64")
        nc.vector.memset(ones64, 1.0)

        mask_sb = []
        for kt in range(KT):
            m = mask_pool.tile([128, s], u8, tag=f"m{kt}", name=f"m{kt}")
            nc.sync.dma_start(m, maskT_d[kt * 128 : (kt + 1) * 128, :])
            mask_sb.append(m)

        qt_sb = [qtkt_pool.tile([128, s], mf, tag=f"q{t}", name=f"qt{t}") for t in range(DT)]
        kt_sb = [qtkt_pool.tile([128, s], mf, tag=f"k{t}", name=f"ktt{t}") for t in range(DT)]
        v_sb = []
        for kt in range(KT):
            v = v_pool.tile([128, hpc * VC], mf, tag=f"v{kt}", name=f"v{kt}")
            nc.sync.dma_start(v, vtpl_d[:, :])
            v_sb.append(v)
        hn_sb = [hn_pool.tile([128, s], mf, tag=f"h{t}", name=f"hn{t}") for t in range(DT)]

        # ---- phase 1: projections
        with ExitStack() as p1:
            x_pool = p1.enter_context(tc.tile_pool(name="xs", bufs=2))
            w_pool = p1.enter_context(tc.tile_pool(name="ws", bufs=ICH))
            xv_pool = p1.enter_context(tc.tile_pool(name="xv", bufs=ICH))
            pp_pool = p1.enter_context(
                tc.tile_pool(name="pp", bufs=2 * DT, space="PSUM")
            )

            def project_dmajor(xT_d, wT_d, out_tiles):
                w_sb = []
                for ic in range(ICH):
                    w = w_pool.tile([128, hpc * dk], mf, tag="w", name="w")
                    nc.sync.dma_start(w, wT_d[ic * 128 : (ic + 1) * 128, :])
                    w_sb.append(w)
                ps = [
                    [pp_pool.tile([128, QW], f32, tag="pp", name="pp") for _ in range(QH)]
                    for _ in range(DT)
                ]
                for ic in range(ICH):
                    xt = x_pool.tile([128, s], mf, tag="x", name="x")
                    nc.sync.dma_start(xt, xT_d[ic * 128 : (ic + 1) * 128, :])
                    for dt in range(DT):
                        for qh in range(QH):
                            for qs in range(QW // 512):
                                q0 = qh * QW + qs * 512
                                nc.tensor.matmul(
                                    ps[dt][qh][:, qs * 512 : (qs + 1) * 512],
                                    w_sb[ic][:, dt * 128 : (dt + 1) * 128],
                                    xt[:, q0 : q0 + 512],
                                    start=(ic == 0),
                                    stop=(ic == ICH - 1),
                                )
                for dt in range(DT):
                    for qh in range(QH):
                        nc.scalar.copy(
                            out_tiles[dt][:, qh * QW : (qh + 1) * QW], ps[dt][qh][:]
                        )

            project_dmajor(xqT_d, wqT_d, qt_sb)
            project_dmajor(xkT_d, wkT_d, kt_sb)

            # V projection: k-major with per-head ones column
            wv_sb = []
            for ic in range(ICH):
                w = w_pool.tile([128, hpc * dk], mf, tag="w", name="wv")
                nc.sync.dma_start(w, wvT_d[ic * 128 : (ic + 1) * 128, :])
                wv_sb.append(w)
            xv_sb = []
            for ic in range(ICH):
                xt = xv_pool.tile([128, s], mf, tag="xv", name="xvt")
                nc.sync.dma_start(xt, xvT_d[ic * 128 : (ic + 1) * 128, :])
                xv_sb.append(xt)
            for kt in range(KT):
                psv = pp_pool.tile([128, hpc * dk], f32, tag="pp", name="psv")
                for ic in range(ICH):
                    nc.tensor.matmul(
                        psv,
                        xv_sb[ic][:, kt * 128 : (kt + 1) * 128],
                        wv_sb[ic],
                        start=(ic == 0),
                        stop=(ic == ICH - 1),
                    )
                for h in range(hpc):
                    nc.vector.tensor_copy(
                        v_sb[kt][:, h * VC : h * VC + dk],
                        psv[:, h * dk : (h + 1) * dk],
                    )

        # ---- phase 2: attention per head
        with ExitStack() as p2:
            pt_pool = p2.enter_context(tc.tile_pool(name="pt", bufs=2))
            sc_pool = p2.enter_context(tc.tile_pool(name="sc", bufs=4, space="PSUM"))
            av_pool = p2.enter_context(tc.tile_pool(name="av", bufs=1, space="PSUM"))
            rc_pool = p2.enter_context(tc.tile_pool(name="rc", bufs=2))
            rep_pool = p2.enter_context(tc.tile_pool(name="rep", bufs=2))

            for h in range(hpc):
                ht, hh = h // 2, h % 2
                av = av_pool.tile([dk + 1, s], f32, tag="av", name="av")
                for kt in range(KT):
                    pt = pt_pool.tile([128, s], mf, tag="pt", name="pt")
                    for qc in range(NQC):
                        ps = sc_pool.tile([128, 512], f32, tag="sc", name="sc")
                        nc.tensor.matmul(
                            ps,
                            kt_sb[ht][hh * dk : (hh + 1) * dk, kt * 128 : (kt + 1) * 128],
                            qt_sb[ht][hh * dk : (hh + 1) * dk, qc * 512 : (qc + 1) * 512],
                            start=True,
                            stop=True,
                        )
                        nc.scalar.activation(
                            pt[:, qc * 512 : (qc + 1) * 512], ps, Exp, scale=scale
                        )
                    nc.vector.tensor_mul(pt, pt, mask_sb[kt])
                    nc.sync.dma_start(attnT_d[h, kt * 128 : (kt + 1) * 128, :], pt)
                    for qc in range(NQC):
                        nc.tensor.matmul(
                            av[:, qc * 512 : (qc + 1) * 512],
                            v_sb[kt][:, h * VC : (h + 1) * VC],
                            pt[:, qc * 512 : (qc + 1) * 512],
                            start=(kt == 0),
                            stop=(kt == KT - 1),
                        )
                rc = rc_pool.tile([1, s], f32, tag="rc", name="rc")
                nc.vector.reciprocal(rc, av[dk : dk + 1, :])
                nc.sync.dma_start(recip_d[h : h + 1, :], rc)
                for qc in range(NQC):
                    rps = sc_pool.tile([128, 512], f32, tag="sc", name="rps")
                    # fp32 (not f32r) so the broadcast is numerically exact
                    nc.tensor.matmul(
                        rps[0:dk, :],
                        ones64,
                        rc[:, qc * 512 : (qc + 1) * 512],
                        start=True,
                        stop=True,
                    )
                    rsb = rep_pool.tile([dk, 512], f32, tag="rep", name="rsb")
                    nc.scalar.copy(rsb, rps[0:dk, :])
                    nc.vector.tensor_mul(
                        hn_sb[ht][hh * dk : (hh + 1) * dk, qc * 512 : (qc + 1) * 512],
                        av[0:dk, qc * 512 : (qc + 1) * 512],
                        rsb,
                    )

        # ---- phase 3: output projection (row-parallel partial)
        with ExitStack() as p3:
            wo_pool = p3.enter_context(tc.tile_pool(name="wo", bufs=1))
            out_pool = p3.enter_context(tc.tile_pool(name="op", bufs=3))
            op_pool = p3.enter_context(tc.tile_pool(name="ops", bufs=2, space="PSUM"))

            wo_sb = []
            for ct in range(DT):
                w = wo_pool.tile([128, d], mf, tag=f"wo{ct}", name=f"wo{ct}")
                nc.sync.dma_start(w, woT_d[ct * 128 : (ct + 1) * 128, :])
                wo_sb.append(w)
            for qt in range(s // 128):
                ot = out_pool.tile([128, d], f32, tag="ot", name="ot")
                for oc in range(NOC):
                    ps = op_pool.tile([128, 512], f32, tag="ops", name="ops")
                    for ct in range(DT):
                        nc.tensor.matmul(
                            ps,
                            hn_sb[ct][:, qt * 128 : (qt + 1) * 128],
                            wo_sb[ct][:, oc * 512 : (oc + 1) * 512],
                            start=(ct == 0),
                            stop=(ct == DT - 1),
                        )
                    nc.scalar.copy(ot[:, oc * 512 : (oc + 1) * 512], ps)
                nc.sync.dma_start(outp_d[qt * 128 : (qt + 1) * 128, :], ot)

    if spill:
        spill_waits(nc)
    return nc


def shard_inputs(query, key, value, mask, key_padding_mask, Wq, Wk, Wv, Wo):
    """Build the 8 per-core input maps from the full inputs."""
    f32 = np.float32
    maskT = {}
    for b in range(B):
        maskT[b] = np.ascontiguousarray(
            (mask.T.astype(np.int32) * key_padding_mask[b][:, None]).astype(np.uint8)
        )
    rnd = round_f32r if F32R else (lambda a: np.ascontiguousarray(a, f32))
    xT = {}
    for b in range(B):
        xT[b] = tuple(
            rnd(np.ascontiguousarray(x[:, b, :].T.astype(f32)))
            for x in (query, key, value)
        )
    in_maps = []
    for c in range(N_CORES):
        b = c // 4
        h0 = (c % 4) * HPC * DK
        sl = slice(h0, h0 + HPC * DK)
        xq, xk, xv = xT[b]
        in_maps.append(
            {
                "xqT": xq,
                "xkT": xk,
                "xvT": xv,
                "wqT": rnd(np.ascontiguousarray(Wq[sl, :].T.astype(f32))),
                "wkT": rnd(np.ascontiguousarray(Wk[sl, :].T.astype(f32))),
                "wvT": rnd(np.ascontiguousarray(Wv[sl, :].T.astype(f32))),
                "woT": rnd(np.ascontiguousarray(Wo[:, sl].T.astype(f32))),
                "maskT": maskT[b],
                "vtpl": np.ones((128, HPC * (DK + 1)), f32),
            }
        )
    return in_maps


def unshard_outputs(results, bo):
    """Assemble full (out, attn) from the 8 per-core result dicts."""
    out = np.zeros((S, B, D), np.float32)
    attn = np.empty((B, H, S, S), np.float32)
    for c, res in enumerate(results):
        b = c // 4
        h0 = (c % 4) * HPC
        out[:, b, :] += res["outp"]
        # attnT: (hpc, k, q) unnormalized; recip: (hpc, q)
        tmp = res["attnT"] * res["recip"][:, None, :]
        for h in range(HPC):
            attn[b, h0 + h] = tmp[h].T
    out += bo.astype(np.float32)
    return out, attn


_NC_CACHE = {}


def kernel(query, key, value, mask, key_padding_mask, Wq, Wk, Wv, Wo, bo):
    from concourse.bass_utils import run_bass_kernel_spmd

    query = np.asarray(query)
    key = np.asarray(key)
    value = np.asarray(value)
    mask = np.asarray(mask)
    key_padding_mask = np.asarray(key_padding_mask)
    Wq, Wk, Wv, Wo, bo = (np.asarray(w) for w in (Wq, Wk, Wv, Wo, bo))

    if "nc" not in _NC_CACHE:
        _NC_CACHE["nc"] = build_nc()
    nc = _NC_CACHE["nc"]

    in_maps = shard_inputs(query, key, value, mask, key_padding_mask, Wq, Wk, Wv, Wo)
    res = run_bass_kernel_spmd(nc, in_maps, core_ids=list(range(N_CORES)))
    return unshard_outputs(res.results, bo)
